# revision 4
# baseline (speedup 1.0000x reference)
"""Trainium2 Bass kernel for the 8x8-block rfft2 magnitude ("DCT") layer.

Computes, for input x [32,1,512,512] f32 and freq_weights [64] f32:
  per 8x8 spatial block: |rfft2(block, norm='ortho')| -> 40 freq bins,
  scaled by sigmoid(freq_weights)[:40], zero-padded to 64 channels.
Output: [32, 64, 64, 64] f32 (channels 40..63 are zero).

Strategy (pure data parallel, 4 images per core on 8 cores):
  The per-block 2D DFT is separable.  Per 128-row x 512-col slab:
    stage 1: one matmul per 128-col chunk with the *data* as the
      stationary operand and a block-diagonal cos/sin matrix streaming:
      out = A_chunk.T @ W1 -> vertical DFT of every row-block, with the
      output already transposed so j (intra-block col) lands on the
      partition axis.
    stage 2: matmuls against block-diagonal cos/sin matrices contract j,
      giving Re/Im of the full 2D DFT laid out [(u,bi), (v,bj)] --
      exactly matching the output tensor's memory order.
  Magnitude + sigmoid-weighting happen on ACT/DVE; a single DMA per slab
  stores 64B..256B-contiguous runs.  Channels 40..63 rely on the runtime
  pre-zeroing ExternalOutput buffers.
"""

import math
import numpy as np
from contextlib import ExitStack

import concourse.bass as bass
import concourse.bacc as bacc
import concourse.mybir as mybir
from concourse import tile
from concourse.bass_utils import run_bass_kernel_spmd

F32 = mybir.dt.float32

N_CORES = 8
IMGS_PER_CORE = 4  # 32 / 8
SLABS_PER_IMG = 4  # 512 rows / 128


def _build_host_matrices(freq_weights: np.ndarray):
    """Block-diagonal DFT coefficient matrices + sigmoid weight tile."""
    # W1 [128, 256]: row p=(bi,i); col n=(reim, u, bi2). Vertical DFT, /8 ortho.
    p = np.arange(128)
    bi_p, i_p = p // 8, p % 8
    n = np.arange(256)
    reim_n, r = n // 128, n % 128
    u_n, bi2_n = r // 16, r % 16
    ang1 = 2.0 * math.pi * np.outer(i_p, u_n) / 8.0  # [128, 256] via broadcast
    W1 = np.where(reim_n[None, :] == 0, np.cos(ang1), np.sin(ang1)) / 8.0
    W1 *= (bi_p[:, None] == bi2_n[None, :])
    W1 = W1.astype(np.float32)

    # C2/S2 [128, 80]: row p=(bj,j); col n=(v, bj2). Horizontal DFT.
    bj_p, j_p = p // 8, p % 8
    m = np.arange(80)
    v_m, bj2_m = m // 16, m % 16
    ang2 = 2.0 * math.pi * np.outer(j_p, v_m) / 8.0
    blk = (bj_p[:, None] == bj2_m[None, :])
    C2 = (np.cos(ang2) * blk).astype(np.float32)
    S2 = (np.sin(ang2) * blk).astype(np.float32)
    S2n = (-S2).astype(np.float32)

    # Wtile [128, 320]: p=(u,bi), f=(v,bj) -> sigmoid(freq_weights)[u*5+v]
    w = 1.0 / (1.0 + np.exp(-freq_weights.astype(np.float64)))
    u_idx = np.arange(128) // 16
    v_idx = np.arange(320) // 64
    Wtile = w[u_idx[:, None] * 5 + v_idx[None, :]].astype(np.float32)
    return W1, C2, S2, S2n, Wtile


_NC_CACHE = None


def _build_bass(n_imgs: int = IMGS_PER_CORE):
    nc = bacc.Bacc("TRN2", target_bir_lowering=False)
    x = nc.dram_tensor("x", [n_imgs * 512, 512], F32, kind="ExternalInput")
    w1 = nc.dram_tensor("w1", [128, 256], F32, kind="ExternalInput")
    c2 = nc.dram_tensor("c2", [128, 80], F32, kind="ExternalInput")
    s2 = nc.dram_tensor("s2", [128, 80], F32, kind="ExternalInput")
    s2n = nc.dram_tensor("s2n", [128, 80], F32, kind="ExternalInput")
    wt = nc.dram_tensor("wt", [128, 320], F32, kind="ExternalInput")
    out = nc.dram_tensor(
        "out", [n_imgs, 64, 64, 64], F32, kind="ExternalOutput"
    )

    # out view for the per-slab store: [img, u, bi_l, s, v, bj]
    out40 = out[:, 0:40, :, :].rearrange(
        "b (u v) (s p) q -> b u p s v q", u=8, v=5, s=SLABS_PER_IMG, p=16
    )

    with tile.TileContext(nc) as tc, ExitStack() as ctx:
        consts = ctx.enter_context(tc.tile_pool(name="consts", bufs=1))
        a_pool = ctx.enter_context(tc.tile_pool(name="a", bufs=3))
        z_pool = ctx.enter_context(tc.tile_pool(name="z", bufs=3))
        sq_pool = ctx.enter_context(tc.tile_pool(name="sq", bufs=2))
        mag_pool = ctx.enter_context(tc.tile_pool(name="mag", bufs=2))
        psz_pool = ctx.enter_context(tc.tile_pool(name="psz", bufs=2, space="PSUM"))
        pso_pool = ctx.enter_context(tc.tile_pool(name="pso", bufs=2, space="PSUM"))

        w1_t = consts.tile([128, 256], F32, tag="w1")
        nc.sync.dma_start(w1_t[:], w1[:])
        c2_t = consts.tile([128, 80], F32, tag="c2")
        nc.sync.dma_start(c2_t[:], c2[:])
        s2_t = consts.tile([128, 80], F32, tag="s2")
        nc.sync.dma_start(s2_t[:], s2[:])
        s2n_t = consts.tile([128, 80], F32, tag="s2n")
        nc.sync.dma_start(s2n_t[:], s2n[:])
        wt_t = consts.tile([128, 320], F32, tag="wt")
        nc.sync.dma_start(wt_t[:], wt[:])

        for img in range(n_imgs):
            for s in range(SLABS_PER_IMG):
                a_t = a_pool.tile([128, 512], F32)
                row0 = img * 512 + s * 128
                nc.sync.dma_start(a_t[:], x[row0 : row0 + 128, :])

                o2re = pso_pool.tile([128, 320], F32, tag="o2re")
                o2im = pso_pool.tile([128, 320], F32, tag="o2im")
                for c in range(4):
                    psz = psz_pool.tile([128, 256], F32, tag="psz")
                    nc.tensor.matmul(
                        psz[:],
                        a_t[:, 128 * c : 128 * (c + 1)],
                        w1_t[:],
                        start=True,
                        stop=True,
                    )
                    z_t = z_pool.tile([128, 256], F32)
                    nc.vector.tensor_copy(z_t[:], psz[:])
                    zre = z_t[:, 0:128]
                    zim = z_t[:, 128:256]
                    sl = slice(80 * c, 80 * (c + 1))
                    nc.tensor.matmul(o2re[:, sl], zre, c2_t[:], start=True, stop=False)
                    nc.tensor.matmul(o2im[:, sl], zre, s2_t[:], start=True, stop=False)
                    nc.tensor.matmul(o2re[:, sl], zim, s2n_t[:], start=False, stop=True)
                    nc.tensor.matmul(o2im[:, sl], zim, c2_t[:], start=False, stop=True)

                sq_re = sq_pool.tile([128, 320], F32, tag="sqre")
                sq_im = sq_pool.tile([128, 320], F32, tag="sqim")
                nc.scalar.square(sq_re[:], o2re[:])
                nc.scalar.square(sq_im[:], o2im[:])
                ssum = sq_pool.tile([128, 320], F32, tag="ssum")
                nc.vector.tensor_add(ssum[:], sq_re[:], sq_im[:])

                # sqrt writes v-major (free = v*64 + 16*c + q) so the store
                # has 64-float contiguous runs per (partition, v).
                root = mag_pool.tile([128, 320], F32, tag="root")
                root_perm = root[:].rearrange("p (v c q) -> p c v q", v=5, c=4, q=16)
                ssum_perm = ssum[:].rearrange("p (c v q) -> p c v q", c=4, v=5, q=16)
                nc.scalar.sqrt(root_perm, ssum_perm)

                magf = mag_pool.tile([128, 320], F32, tag="magf")
                nc.vector.tensor_mul(magf[:], root[:], wt_t[:])
                # per-v store: DRAM side merges (bi_l, bj) into 4KB runs
                for v in range(5):
                    nc.sync.dma_start(
                        out40[img, :, :, s, v, :], magf[:, 64 * v : 64 * (v + 1)]
                    )
    nc.finalize()
    return nc


def kernel(x: np.ndarray, freq_weights: np.ndarray) -> np.ndarray:
    global _NC_CACHE
    x = np.ascontiguousarray(np.asarray(x, dtype=np.float32))
    freq_weights = np.asarray(freq_weights, dtype=np.float32)
    B = x.shape[0]
    assert x.shape == (32, 1, 512, 512) and freq_weights.shape == (64,)

    W1, C2, S2, S2n, Wtile = _build_host_matrices(freq_weights)
    if _NC_CACHE is None:
        _NC_CACHE = _build_bass()
    nc = _NC_CACHE

    per = B // N_CORES
    in_maps = []
    for k in range(N_CORES):
        in_maps.append(
            {
                "x": x[k * per : (k + 1) * per].reshape(per * 512, 512),
                "w1": W1,
                "c2": C2,
                "s2": S2,
                "s2n": S2n,
                "wt": Wtile,
            }
        )
    res = run_bass_kernel_spmd(nc, in_maps, list(range(N_CORES))).results
    out = np.concatenate([res[k]["out"] for k in range(N_CORES)], axis=0)
    return out.astype(np.float32)


# revision 5
# speedup vs baseline: 7.4244x; 7.4244x over previous
"""Trainium2 Bass kernel for the 8x8-block rfft2 magnitude ("DCT") layer.

Computes, for input x [32,1,512,512] f32 and freq_weights [64] f32:
  per 8x8 spatial block: |rfft2(block, norm='ortho')| -> 40 freq bins,
  scaled by sigmoid(freq_weights)[:40], zero-padded to 64 channels.
Output: [32, 64, 64, 64] f32 (channels 40..63 are zero).

Strategy (pure data parallel, 4 images per core on 8 cores):
  The per-block 2D DFT is separable.  Per 128-row x 512-col slab:
    stage 1: one matmul per 128-col chunk with the *data* as the
      stationary operand and a block-diagonal cos/sin matrix streaming:
      out = A_chunk.T @ W1 -> vertical DFT of every row-block, with the
      output already transposed so j (intra-block col) lands on the
      partition axis.
    stage 2: matmuls against block-diagonal cos/sin matrices contract j,
      giving Re/Im of the full 2D DFT laid out [(u,bi), (v,bj)] --
      exactly matching the output tensor's memory order.
  Magnitude + sigmoid-weighting happen on ACT/DVE; a single DMA per slab
  stores 64B..256B-contiguous runs.  Channels 40..63 rely on the runtime
  pre-zeroing ExternalOutput buffers.
"""

import math
import numpy as np
from contextlib import ExitStack

import concourse.bass as bass
import concourse.bacc as bacc
import concourse.mybir as mybir
from concourse import tile
from concourse.bass_utils import run_bass_kernel_spmd

F32 = mybir.dt.float32

N_CORES = 8
IMGS_PER_CORE = 4  # 32 / 8
SLABS_PER_IMG = 4  # 512 rows / 128


def _build_host_matrices(freq_weights: np.ndarray):
    """Block-diagonal DFT coefficient matrices + sigmoid weight tile."""
    # W1 [128, 256]: row p=(bi,i); col n=(reim, u, bi2). Vertical DFT, /8 ortho.
    p = np.arange(128)
    bi_p, i_p = p // 8, p % 8
    n = np.arange(256)
    reim_n, r = n // 128, n % 128
    u_n, bi2_n = r // 16, r % 16
    ang1 = 2.0 * math.pi * np.outer(i_p, u_n) / 8.0  # [128, 256] via broadcast
    W1 = np.where(reim_n[None, :] == 0, np.cos(ang1), np.sin(ang1)) / 8.0
    W1 *= (bi_p[:, None] == bi2_n[None, :])
    W1 = W1.astype(np.float32)

    # C2/S2 [128, 80]: row p=(bj,j); col n=(v, bj2). Horizontal DFT.
    bj_p, j_p = p // 8, p % 8
    m = np.arange(80)
    v_m, bj2_m = m // 16, m % 16
    ang2 = 2.0 * math.pi * np.outer(j_p, v_m) / 8.0
    blk = (bj_p[:, None] == bj2_m[None, :])
    C2 = (np.cos(ang2) * blk).astype(np.float32)
    S2 = (np.sin(ang2) * blk).astype(np.float32)
    S2n = (-S2).astype(np.float32)

    # Wtile [128, 320]: p=(u,bi), f=(v,bj) -> sigmoid(freq_weights)[u*5+v]
    w = 1.0 / (1.0 + np.exp(-freq_weights.astype(np.float64)))
    u_idx = np.arange(128) // 16
    v_idx = np.arange(320) // 64
    Wtile = w[u_idx[:, None] * 5 + v_idx[None, :]].astype(np.float32)
    return W1, C2, S2, S2n, Wtile


_NC_CACHE = None


def _build_bass(n_imgs: int = IMGS_PER_CORE, repeat: int = 1):
    nc = bacc.Bacc("TRN2", target_bir_lowering=False)
    x = nc.dram_tensor("x", [n_imgs * 512, 512], F32, kind="ExternalInput")
    w1 = nc.dram_tensor("w1", [128, 256], F32, kind="ExternalInput")
    c2 = nc.dram_tensor("c2", [128, 80], F32, kind="ExternalInput")
    s2 = nc.dram_tensor("s2", [128, 80], F32, kind="ExternalInput")
    s2n = nc.dram_tensor("s2n", [128, 80], F32, kind="ExternalInput")
    wt = nc.dram_tensor("wt", [128, 320], F32, kind="ExternalInput")
    out = nc.dram_tensor(
        "out", [n_imgs, 64, 64, 64], F32, kind="ExternalOutput"
    )

    # out view for the per-slab store: [img, u, bi_l, s, v, bj]
    out40 = out[:, 0:40, :, :].rearrange(
        "b (u v) (s p) q -> b u p s v q", u=8, v=5, s=SLABS_PER_IMG, p=16
    )

    with tile.TileContext(nc) as tc, ExitStack() as ctx:
        consts = ctx.enter_context(tc.tile_pool(name="consts", bufs=1))
        a_pool = ctx.enter_context(tc.tile_pool(name="a", bufs=3))
        z_pool = ctx.enter_context(tc.tile_pool(name="z", bufs=3))
        sq_pool = ctx.enter_context(tc.tile_pool(name="sq", bufs=2))
        mag_pool = ctx.enter_context(tc.tile_pool(name="mag", bufs=2))
        psz_pool = ctx.enter_context(tc.tile_pool(name="psz", bufs=2, space="PSUM"))
        pso_pool = ctx.enter_context(tc.tile_pool(name="pso", bufs=2, space="PSUM"))

        w1_t = consts.tile([128, 256], F32, tag="w1")
        nc.sync.dma_start(w1_t[:], w1[:])
        c2_t = consts.tile([128, 80], F32, tag="c2")
        nc.sync.dma_start(c2_t[:], c2[:])
        s2_t = consts.tile([128, 80], F32, tag="s2")
        nc.sync.dma_start(s2_t[:], s2[:])
        s2n_t = consts.tile([128, 80], F32, tag="s2n")
        nc.sync.dma_start(s2n_t[:], s2n[:])
        wt_t = consts.tile([128, 320], F32, tag="wt")
        nc.sync.dma_start(wt_t[:], wt[:])

        rep_ctx = tc.For_i(0, repeat, 1) if repeat > 1 else None
        if rep_ctx is not None:
            rep_ctx.__enter__()
        for img in range(n_imgs):
            for s in range(SLABS_PER_IMG):
                a_t = a_pool.tile([128, 512], F32)
                row0 = img * 512 + s * 128
                nc.sync.dma_start(a_t[:], x[row0 : row0 + 128, :])

                o2re = pso_pool.tile([128, 320], F32, tag="o2re")
                o2im = pso_pool.tile([128, 320], F32, tag="o2im")
                for c in range(4):
                    psz = psz_pool.tile([128, 256], F32, tag="psz")
                    nc.tensor.matmul(
                        psz[:],
                        a_t[:, 128 * c : 128 * (c + 1)],
                        w1_t[:],
                        start=True,
                        stop=True,
                    )
                    z_t = z_pool.tile([128, 256], F32)
                    nc.vector.tensor_copy(z_t[:], psz[:])
                    zre = z_t[:, 0:128]
                    zim = z_t[:, 128:256]
                    sl = slice(80 * c, 80 * (c + 1))
                    nc.tensor.matmul(o2re[:, sl], zre, c2_t[:], start=True, stop=False)
                    nc.tensor.matmul(o2im[:, sl], zre, s2_t[:], start=True, stop=False)
                    nc.tensor.matmul(o2re[:, sl], zim, s2n_t[:], start=False, stop=True)
                    nc.tensor.matmul(o2im[:, sl], zim, c2_t[:], start=False, stop=True)

                sq_re = sq_pool.tile([128, 320], F32, tag="sqre")
                sq_im = sq_pool.tile([128, 320], F32, tag="sqim")
                nc.scalar.square(sq_re[:], o2re[:])
                nc.scalar.square(sq_im[:], o2im[:])
                ssum = sq_pool.tile([128, 320], F32, tag="ssum")
                nc.vector.tensor_add(ssum[:], sq_re[:], sq_im[:])

                # sqrt writes v-major (free = v*64 + 16*c + q) so the store
                # has 64-float contiguous runs per (partition, v).
                root = mag_pool.tile([128, 320], F32, tag="root")
                root_perm = root[:].rearrange("p (v c q) -> p c v q", v=5, c=4, q=16)
                ssum_perm = ssum[:].rearrange("p (c v q) -> p c v q", c=4, v=5, q=16)
                nc.scalar.sqrt(root_perm, ssum_perm)

                magf = mag_pool.tile([128, 320], F32, tag="magf")
                nc.vector.tensor_mul(magf[:], root[:], wt_t[:])
                # per-v store: DRAM side merges (bi_l, bj) into 4KB runs
                for v in range(5):
                    nc.sync.dma_start(
                        out40[img, :, :, s, v, :], magf[:, 64 * v : 64 * (v + 1)]
                    )
        if rep_ctx is not None:
            rep_ctx.__exit__(None, None, None)
    nc.finalize()
    return nc


def kernel(x: np.ndarray, freq_weights: np.ndarray) -> np.ndarray:
    global _NC_CACHE
    x = np.ascontiguousarray(np.asarray(x, dtype=np.float32))
    freq_weights = np.asarray(freq_weights, dtype=np.float32)
    B = x.shape[0]
    assert x.shape == (32, 1, 512, 512) and freq_weights.shape == (64,)

    W1, C2, S2, S2n, Wtile = _build_host_matrices(freq_weights)
    if _NC_CACHE is None:
        _NC_CACHE = _build_bass()
    nc = _NC_CACHE

    per = B // N_CORES
    in_maps = []
    for k in range(N_CORES):
        in_maps.append(
            {
                "x": x[k * per : (k + 1) * per].reshape(per * 512, 512),
                "w1": W1,
                "c2": C2,
                "s2": S2,
                "s2n": S2n,
                "wt": Wtile,
            }
        )
    res = run_bass_kernel_spmd(nc, in_maps, list(range(N_CORES))).results
    out = np.concatenate([res[k]["out"] for k in range(N_CORES)], axis=0)
    return out.astype(np.float32)


# revision 21
# speedup vs baseline: 21.6866x; 2.9210x over previous
"""Trainium2 Bass kernel for the 8x8-block rfft2 magnitude ("DCT") layer.

Computes, for input x [32,1,512,512] f32 and freq_weights [64] f32:
  per 8x8 spatial block: |rfft2(block, norm='ortho')| -> 40 freq bins,
  scaled by sigmoid(freq_weights)[:40], zero-padded to 64 channels.
Output: [32, 64, 64, 64] f32 (channels 40..63 are zero).

Strategy (pure data parallel, 4 images per core on 8 cores):
  The per-block 2D DFT is separable.  Per 128-row x 512-col slab:
    stage 1 (one matmul per 128-col chunk): data is the *stationary*
      operand, a block-diagonal cos/sin matrix streams:
      Z = A_chunk.T @ W1 -> vertical DFT of every row-block with the
      output transposed so j (intra-block col) is on partitions.
    stage 2 (two accumulating matmuls per chunk): Z_re/Z_im stationary,
      [C2|S2|0] / [-S2|C2|0] streaming -> Re/Im of the 2D DFT laid out
      [(bi,u), (v,bj)], matching output memory order after (u,v) merge.
  Matmul operands use float32r (TF32-class, ~1e-4 rel err, 4x rate at
  N>=256).  Magnitude on DVE (squares/add) + ACT (sqrt only, no table
  swap), sigmoid-weighting on DVE, one store DMA per slab with 256B
  runs.  Channels 40..63 rely on the runtime pre-zeroing outputs.
"""

import math
import numpy as np
from contextlib import ExitStack

import concourse.bass as bass
import concourse.bacc as bacc
import concourse.mybir as mybir
from concourse import tile
from concourse.bass_utils import run_bass_kernel_spmd

F32 = mybir.dt.float32
F32R = mybir.dt.float32r

N_CORES = 8
IMGS_PER_CORE = 4  # 32 / 8
SLABS_PER_IMG = 4  # 512 rows / 128


def _build_host_matrices(freq_weights: np.ndarray):
    """Block-diagonal DFT coefficient matrices + sigmoid weight tile."""
    p = np.arange(128)
    # W1 [128, 256]: row p=(bi,i); col n=(reim, bi2, u). Vertical DFT, /8.
    bi_p, i_p = p // 8, p % 8
    n = np.arange(256)
    reim_n, r = n // 128, n % 128
    bi2_n, u_n = r // 8, r % 8
    ang1 = 2.0 * math.pi * np.outer(i_p, u_n) / 8.0
    W1 = np.where(reim_n[None, :] == 0, np.cos(ang1), np.sin(ang1)) / 8.0
    W1 *= (bi_p[:, None] == bi2_n[None, :])
    W1 = W1.astype(np.float32)

    # C2/S2 [128, 80]: row p=(bj,j); col m=(v, bj2). Horizontal DFT.
    bj_p, j_p = p // 8, p % 8
    m = np.arange(80)
    v_m, bj2_m = m // 16, m % 16
    ang2 = 2.0 * math.pi * np.outer(j_p, v_m) / 8.0
    blk = (bj_p[:, None] == bj2_m[None, :])
    C2 = (np.cos(ang2) * blk).astype(np.float32)
    S2 = (np.sin(ang2) * blk).astype(np.float32)
    z96 = np.zeros((128, 96), dtype=np.float32)
    # padded to N=256 so float32r streams at 1 cycle/row
    CS2P = np.concatenate([C2, S2, z96], axis=1)
    SNC2P = np.concatenate([-S2, C2, z96], axis=1)

    # Wtile [128, 320]: p=(bi,u), f=(v,bj) -> sigmoid(freq_weights)[u*5+v]
    w = 1.0 / (1.0 + np.exp(-freq_weights.astype(np.float64)))
    u_idx = np.arange(128) % 8
    v_idx = np.arange(320) // 64
    Wtile = w[u_idx[:, None] * 5 + v_idx[None, :]].astype(np.float32)
    return W1, CS2P, SNC2P, Wtile


_NC_CACHE = None


def _build_bass(n_imgs: int = IMGS_PER_CORE, repeat: int = 1):
    nc = bacc.Bacc("TRN2", target_bir_lowering=False)
    x = nc.dram_tensor("x", [n_imgs * 512, 512], F32R, kind="ExternalInput")
    cst = nc.dram_tensor("cst", [128, 1088], F32R, kind="ExternalInput")
    out = nc.dram_tensor(
        "out", [n_imgs, 64, 64, 64], F32, kind="ExternalOutput"
    )

    # store view: [img, bi_l, s, u, v, bj]; (u,v) merges into one AP dim
    out40 = out[:, 0:40, :, :].rearrange(
        "b (u v) (s p) q -> b p s u v q", u=8, v=5, s=SLABS_PER_IMG, p=16
    )

    with tile.TileContext(nc) as tc, ExitStack() as ctx:
        consts = ctx.enter_context(tc.tile_pool(name="consts", bufs=1))
        a_pool = ctx.enter_context(tc.tile_pool(name="a", bufs=8))
        z_pool = ctx.enter_context(tc.tile_pool(name="z", bufs=12))
        sq_pool = ctx.enter_context(tc.tile_pool(name="sq", bufs=8))
        mag_pool = ctx.enter_context(tc.tile_pool(name="mag", bufs=8))
        psz_pool = ctx.enter_context(tc.tile_pool(name="psz", bufs=3, space="PSUM"))
        pso_pool = ctx.enter_context(tc.tile_pool(name="pso", bufs=5, space="PSUM"))

        cst_t = consts.tile([128, 1088], F32R, tag="cst")
        nc.sync.dma_start(cst_t[:], cst[:])
        w1_t = cst_t[:, 0:256]
        cs2_t = cst_t[:, 256:512]
        snc2_t = cst_t[:, 512:768]
        wt_t = cst_t[:, 768:1088]

        rep_ctx = tc.For_i(0, repeat, 1) if repeat > 1 else None
        if rep_ctx is not None:
            rep_ctx.__enter__()
        for img in range(n_imgs):
            for s in range(SLABS_PER_IMG):
                a_t = a_pool.tile([128, 512], F32R)
                row0 = img * 512 + s * 128
                nc.scalar.dma_start(a_t[:], x[row0 : row0 + 128, :])

                root = mag_pool.tile([128, 320], F32, tag="root")
                sq = sq_pool.tile([128, 640], F32, tag="sq")
                # stage 1 for all chunks first: PE never stalls on the
                # DVE copy of the same chunk's Z
                zts = []
                for c in range(4):
                    psz = psz_pool.tile([128, 256], F32, tag="psz")
                    nc.tensor.matmul(
                        psz[:],
                        a_t[:, 128 * c : 128 * (c + 1)],
                        w1_t,
                        start=True,
                        stop=True,
                    )
                    z_t = z_pool.tile([128, 256], F32R)
                    nc.vector.tensor_copy(z_t[:], psz[:])
                    zts.append(z_t)
                for c in range(4):
                    z_t = zts[c]
                    zre = z_t[:, 0:128]
                    zim = z_t[:, 128:256]
                    o2 = pso_pool.tile([128, 256], F32, tag="o2")
                    nc.tensor.matmul(o2[:], zre, cs2_t, start=True, stop=False)
                    nc.tensor.matmul(o2[:], zim, snc2_t, start=False, stop=True)
                    # square on ACT into the slab-level sq tile
                    nc.scalar.square(sq[:, 160 * c : 160 * (c + 1)], o2[:, 0:160])

                # one add / sqrt / weight-mul per slab (batched over chunks)
                ssum = sq_pool.tile([128, 320], F32, tag="ssum")
                sqv = sq[:].rearrange("p (c h g) -> p c h g", c=4, h=2, g=80)
                nc.vector.tensor_add(
                    ssum[:].rearrange("p (c g) -> p c g", c=4, g=80),
                    sqv[:, :, 0],
                    sqv[:, :, 1],
                )
                # write v-major into root: free = v*64 + 16*c + (0..16)
                nc.scalar.sqrt(
                    root[:].rearrange("p (v c q) -> p c v q", v=5, c=4, q=16),
                    ssum[:].rearrange("p (c v q) -> p c v q", c=4, v=5, q=16),
                )
                magf = mag_pool.tile([128, 320], F32, tag="magf")
                nc.gpsimd.tensor_mul(magf[:], root[:], wt_t)
                nc.sync.dma_start(out40[img, :, s], magf[:])
        if rep_ctx is not None:
            rep_ctx.__exit__(None, None, None)
    nc.finalize()
    return nc


def kernel(x: np.ndarray, freq_weights: np.ndarray) -> np.ndarray:
    global _NC_CACHE
    x = np.ascontiguousarray(np.asarray(x, dtype=np.float32))
    freq_weights = np.asarray(freq_weights, dtype=np.float32)
    B = x.shape[0]
    assert x.shape == (32, 1, 512, 512) and freq_weights.shape == (64,)

    W1, CS2P, SNC2P, Wtile = _build_host_matrices(freq_weights)
    cst = np.concatenate([W1, CS2P, SNC2P, Wtile], axis=1)
    if _NC_CACHE is None:
        _NC_CACHE = _build_bass()
    nc = _NC_CACHE

    per = B // N_CORES
    in_maps = []
    for k in range(N_CORES):
        in_maps.append(
            {
                "x": x[k * per : (k + 1) * per].reshape(per * 512, 512),
                "cst": cst,
            }
        )
    res = run_bass_kernel_spmd(nc, in_maps, list(range(N_CORES))).results
    out = np.concatenate([res[k]["out"] for k in range(N_CORES)], axis=0)
    return out.astype(np.float32)
